# revision 16
# baseline (speedup 1.0000x reference)
"""Trainium2 Bass kernel for a 2-layer LSTM motion-prior sampler (DBSE).

Computes, per time step t (B=2048, T=64, Z=64, H=512):
    h1,c1 = LSTMCell(x_t, h1, c1; W_ih1, W_hh1, b1)     x_t = z_post[:, t-1] (0 at t=0)
    h2,c2 = LSTMCell(h1, h2, c2; W_ih2, W_hh2, b2)
    m_t   = h2 @ W_mean.T + b_mean
    lv_t  = h2 @ W_logvar.T + b_logvar
    z_t   = m_t + eps_t * exp(0.5 * lv_t)
Returns (z_means, z_logvars, z_out), each [B, T, Z] float32.

Sharding: data-parallel over batch across 8 NeuronCores (256 rows each),
weights replicated, recurrence local per core, no collectives.

Device layout: everything feature-major [feature, batch] so LSTM matmul
outputs land directly in recurrence layout with zero transposes.
Matmul inputs are bf16 (fp32 PSUM accumulation); c-state and all
elementwise math stay fp32. Biases are folded into the matmuls (ones-row
trick for layer 1, K=1 bias matmuls for layer 2 / heads).

exp(0.5*lv) is computed in-loop as s/(1-s) with s = sigmoid(0.5*lv) so
the ScalarEngine keeps a single sigmoid/tanh activation-table set for
the whole kernel (an exp table switch costs ~2.7us and would serialize).

PSUM: one 2-bank tile per hidden k-tile ([i|f|o|g]*256), 3 rotating
slots, plus a dedicated 1-bank pool for the mean|logvar head. All
biases enter as K=65 matmuls against the [x;1] rhs with zero-padded
weight rows (K=1 bias matmuls cost a ~400ns PE pipeline break each).
Layer-2 is emitted h2-source-major with the h1-dependent k-slices last
and kc-major across tiles, so the PE absorbs the layer-1 elementwise
latency instead of stalling.
"""

import os
import sys

for _p in ("/opt/trn_rl_repo", "/root/.axon_site/_ro/trn_rl_repo"):
    if os.path.isdir(_p) and _p not in sys.path:
        sys.path.insert(0, _p)

import numpy as np
import ml_dtypes

B = 2048
T = int(os.environ.get("KERNEL_T", "64"))
Z = 64
H = 512
G = 4 * H           # 2048 gate units per layer
NCORES = 8
BL = B // NCORES    # 256 batch rows per core
KT = H // 128       # 4 contraction k-tiles of 128
BF = ml_dtypes.bfloat16

_CACHE = {}


def _build_bass():
    import concourse.tile as tile
    from concourse import bacc, mybir

    f32 = mybir.dt.float32
    bf = mybir.dt.bfloat16
    AF = mybir.ActivationFunctionType
    OP = mybir.AluOpType

    nc = bacc.Bacc(None, target_bir_lowering=False)

    # ---------------- DRAM I/O (per core) ----------------
    xT = nc.dram_tensor("xT", [Z + 1, T * BL], bf, kind="ExternalInput")
    epsT = nc.dram_tensor("epsT", [T, Z, BL], f32, kind="ExternalInput")
    w1x_d = nc.dram_tensor("w1x", [Z + 1, G], bf, kind="ExternalInput")
    whh1_d = nc.dram_tensor("whh1", [H, G], bf, kind="ExternalInput")
    wih2_d = nc.dram_tensor("wih2", [H, G], bf, kind="ExternalInput")
    whh2_d = nc.dram_tensor("whh2", [H, G], bf, kind="ExternalInput")
    whd_d = nc.dram_tensor("whd", [H, 2 * Z], bf, kind="ExternalInput")
    b2_d = nc.dram_tensor("b2", [Z + 1, G], bf, kind="ExternalInput")
    bhd_d = nc.dram_tensor("bhd", [Z + 1, 2 * Z], bf, kind="ExternalInput")
    zm_d = nc.dram_tensor("zm", [T, Z, BL], f32, kind="ExternalOutput")
    zlv_d = nc.dram_tensor("zlv", [T, Z, BL], f32, kind="ExternalOutput")
    zz_d = nc.dram_tensor("zz", [T, Z, BL], f32, kind="ExternalOutput")

    # psum chunk position -> weight-column base; gates ordered [i,f,o | g]
    # in PSUM so one sigmoid covers [0:1536] and one tanh covers [1536:2048].
    # PyTorch weight rows are [i, f, g, o] blocks of H.
    GBASE = [0 * H, 1 * H, 3 * H, 2 * H]  # i, f, o, g

    with tile.TileContext(nc) as tc:
        from contextlib import ExitStack

        with ExitStack() as ctx:
            state = ctx.enter_context(tc.tile_pool(name="state", bufs=1))

            # ---------------- persistent SBUF ----------------
            # order matters: the t=0 layer-1 matmuls only need w1x/x/whh1,
            # so those transfers go first and the loop starts sooner.
            w1x = state.tile([Z + 1, G], bf)
            nc.sync.dma_start(w1x[:], w1x_d[:])
            xall = state.tile([Z + 1, T, BL], bf)
            nc.sync.dma_start(xall[:], xT.rearrange("p (t b) -> p t b", t=T))
            whh1 = state.tile([128, KT, G], bf)
            for kc in range(KT):
                eng = nc.sync if kc % 2 == 0 else nc.scalar
                eng.dma_start(whh1[:, kc, :], whh1_d[kc * 128 : (kc + 1) * 128, :])
            whh2 = state.tile([128, KT, G], bf)
            for kc in range(KT):
                eng = nc.sync if kc % 2 == 0 else nc.scalar
                eng.dma_start(whh2[:, kc, :], whh2_d[kc * 128 : (kc + 1) * 128, :])
            wih2 = state.tile([128, KT, G], bf)
            for kc in range(KT):
                eng = nc.sync if kc % 2 == 0 else nc.scalar
                eng.dma_start(wih2[:, kc, :], wih2_d[kc * 128 : (kc + 1) * 128, :])
            whd = state.tile([128, KT, 2 * Z], bf)
            nc.sync.dma_start(whd[:], whd_d.rearrange("(k p) g -> p k g", p=128))
            b2 = state.tile([Z + 1, G], bf)
            nc.sync.dma_start(b2[:], b2_d[:])
            bhd = state.tile([Z + 1, 2 * Z], bf)
            nc.sync.dma_start(bhd[:], bhd_d[:])

            # double-buffered h (bf16, matmul input); single-buffer c (fp32)
            h1b0 = state.tile([128, KT * BL], bf)
            h1b1 = state.tile([128, KT * BL], bf)
            h2b0 = state.tile([128, KT * BL], bf)
            h2b1 = state.tile([128, KT * BL], bf)
            c1 = state.tile([128, KT * BL], f32)
            c2 = state.tile([128, KT * BL], f32)
            for buf in (h1b0, h1b1, h2b0, h2b1):
                nc.gpsimd.memset(buf[:], 0.0)
            nc.gpsimd.memset(c1[:], 0.0)
            nc.gpsimd.memset(c2[:], 0.0)
            h1 = (h1b0, h1b1)
            h2 = (h2b0, h2b1)

            with (
                tc.tile_pool(name="psum", bufs=3, space="PSUM") as psum,
                tc.tile_pool(name="psum_hd", bufs=2, space="PSUM") as psum_hd,
                tc.tile_pool(name="work", bufs=4) as work,
                tc.tile_pool(name="epsp", bufs=4) as epsp,
            ):
                # One PSUM tile per hidden k-tile (2 banks): [i|f|o|g]*256.
                # Bank A = [0:512] (i,f), bank B = [512:1024] (o,g), so the
                # start-matmul pairs (i,o) then (f,g) are bank-disjoint and
                # row-strip-packed K=1 bias matmuls run concurrently.
                POS_OFF = {"i": 0, "f": 256, "o": 512, "g": 768}
                POS_COL = {"i": 0, "f": H, "o": 3 * H, "g": 2 * H}
                POS_ROW = {"i": 0, "o": 32, "f": 64, "g": 96}
                GROUPS = (("i", "o"), ("f", "g"))

                def l1_group(t, k, pg, h_cur, grp):
                    """x-gate starts + W_hh1 accumulation for chunk pair grp
                    of k-tile k (layer 1)."""
                    for pos in grp:
                        off = POS_OFF[pos]
                        col = POS_COL[pos] + k * 128
                        nc.tensor.matmul(
                            pg[:, off : off + 256],
                            w1x[:, col : col + 128],
                            xall[:, t, :],
                            start=True,
                            stop=False,
                        )
                    for kc in range(KT):
                        for pos in grp:
                            off = POS_OFF[pos]
                            col = POS_COL[pos] + k * 128
                            nc.tensor.matmul(
                                pg[:, off : off + 256],
                                whh1[:, kc, col : col + 128],
                                h_cur[:, kc * 256 : (kc + 1) * 256],
                                start=False,
                                stop=(kc == KT - 1),
                            )

                def l2_pre(t, k, pg, h2_cur, grp):
                    """row-packed K=1 bias starts + the h2-dependent
                    accumulation for chunk pair grp of k-tile k (layer 2)."""
                    for pos in grp:
                        off = POS_OFF[pos]
                        col = POS_COL[pos] + k * 128
                        nc.tensor.matmul(
                            pg[:, off : off + 256],
                            b2[:, col : col + 128],
                            xall[:, t, :],
                            start=True,
                            stop=False,
                        )
                    for kc in range(KT):
                        for pos in grp:
                            off = POS_OFF[pos]
                            col = POS_COL[pos] + k * 128
                            nc.tensor.matmul(
                                pg[:, off : off + 256],
                                whh2[:, kc, col : col + 128],
                                h2_cur[:, kc * 256 : (kc + 1) * 256],
                                start=False,
                                stop=False,
                            )

                def l2_h1(t, k, pg, h1_new, grp, kc):
                    """one h1-dependent k-slice for chunk pair grp; emitted
                    kc-major across tiles so each h1 k-tile is consumed as
                    soon as the layer-1 elementwise produces it."""
                    for pos in grp:
                        off = POS_OFF[pos]
                        col = POS_COL[pos] + k * 128
                        nc.tensor.matmul(
                            pg[:, off : off + 256],
                            wih2[:, kc, col : col + 128],
                            h1_new[:, kc * 256 : (kc + 1) * 256],
                            start=False,
                            stop=(kc == KT - 1),
                        )

                def eltwise(t, lname, pgs, c, h_new):
                    """Per-k-tile LSTM cell elementwise, software-pipelined:
                    ACT order s0 g0 s1 g1 th0 s2 g2 th1 s3 g3 th2 th3 keeps
                    the scalar engine busy while the DVE runs the c-updates,
                    and each h k-tile is published as early as possible."""
                    ifo = [None] * KT
                    gg = [None] * KT
                    th = [None] * KT

                    def h_mul(k):
                        nc.vector.tensor_mul(
                            h_new[:, k * 256 : (k + 1) * 256],
                            ifo[k][:, 512:768],
                            th[k][:],
                        )

                    def tanh_c(k):
                        th[k] = work.tile(
                            [128, 256], bf, tag="th", name=f"th_{lname}_{t}_{k}"
                        )
                        nc.scalar.activation(
                            th[k][:], c[:, k * 256 : (k + 1) * 256], AF.Tanh
                        )

                    for k in range(KT):
                        ifo[k] = work.tile(
                            [128, 768], f32, tag="ifo", name=f"ifo_{lname}_{t}_{k}"
                        )
                        nc.scalar.activation(ifo[k][:], pgs[k][:, 0:768], AF.Sigmoid)
                        gg[k] = work.tile(
                            [128, 256], f32, tag="gg", name=f"gg_{lname}_{t}_{k}"
                        )
                        nc.scalar.activation(gg[k][:], pgs[k][:, 768:1024], AF.Tanh)
                        if k >= 2:
                            h_mul(k - 2)
                        cs = c[:, k * 256 : (k + 1) * 256]
                        t1 = work.tile([128, 256], f32, tag="t1", name=f"t1_{lname}_{t}_{k}")
                        nc.vector.tensor_mul(t1[:], ifo[k][:, 256:512], cs)
                        t2 = work.tile([128, 256], f32, tag="t2", name=f"t2_{lname}_{t}_{k}")
                        nc.vector.tensor_mul(t2[:], ifo[k][:, 0:256], gg[k][:])
                        nc.vector.tensor_add(cs, t1[:], t2[:])
                        if k >= 1:
                            tanh_c(k - 1)
                    tanh_c(KT - 1)
                    h_mul(KT - 2)
                    h_mul(KT - 1)

                def heads_mm(t, h2_new):
                    """mean|logvar head matmuls for step t -> SBUF copy."""
                    ph = psum_hd.tile([128, BL], f32, tag="hd", name=f"ph_{t}")
                    nc.tensor.matmul(
                        ph[:, 0:BL], bhd[:], xall[:, t, :], start=True, stop=False
                    )
                    for kc in range(KT):
                        nc.tensor.matmul(
                            ph[:, 0:BL],
                            whd[:, kc, :],
                            h2_new[:, kc * 256 : (kc + 1) * 256],
                            start=False,
                            stop=(kc == KT - 1),
                        )
                    mlv = work.tile([128, BL], f32, tag="mlv", name=f"mlv_{t}")
                    nc.vector.tensor_copy(mlv[:], ph[:, 0:BL])
                    return mlv

                def heads_z(t, mlv):
                    """z_t = m + eps * exp(0.5*lv), with exp via the
                    sigmoid table set: exp(x) = s/(1-s), s = sigmoid(x/2).
                    lv lives on partitions 64..127; a small SBUF->SBUF DMA
                    aligns it with m on partitions 0..63."""
                    lvs = work.tile([Z, BL], f32, tag="lvs", name=f"lvs_{t}")
                    nc.sync.dma_start(lvs[:], mlv[Z : 2 * Z, :])
                    s = work.tile([Z, BL], f32, tag="s", name=f"s_{t}")
                    nc.scalar.activation(s[:], lvs[:], AF.Sigmoid, scale=0.5)
                    u = work.tile([Z, BL], f32, tag="u", name=f"u_{t}")
                    nc.vector.tensor_scalar(u[:], s[:], -1.0, 1.0, OP.mult, OP.add)
                    r = work.tile([Z, BL], f32, tag="r", name=f"r_{t}")
                    nc.vector.reciprocal(r[:], u[:])
                    e = work.tile([Z, BL], f32, tag="e", name=f"e_{t}")
                    nc.vector.tensor_mul(e[:], s[:], r[:])
                    epst = epsp.tile([Z, BL], f32, tag="eps", name=f"eps_{t}")
                    nc.sync.dma_start(epst[:], epsT[t])
                    zt = work.tile([Z, BL], f32, tag="zt", name=f"zt_{t}")
                    nc.gpsimd.tensor_mul(zt[:], e[:], epst[:])
                    nc.gpsimd.tensor_add(zt[:], zt[:], mlv[0:Z, :])
                    nc.sync.dma_start(zm_d[t], mlv[0:Z, :])
                    nc.sync.dma_start(zlv_d[t], lvs[:])
                    nc.sync.dma_start(zz_d[t], zt[:])

                # ---------------- recurrence ----------------
                for t in range(T):
                    cur, nxt = t % 2, (t + 1) % 2
                    pg1 = [
                        psum.tile([128, 1024], f32, tag="g", name=f"p1_{t}_{k}")
                        for k in range(KT)
                    ]
                    for k in range(KT):
                        for grp in GROUPS:
                            l1_group(t, k, pg1[k], h1[cur], grp)
                    eltwise(t, "l1", pg1, c1, h1[nxt])
                    pg2 = [
                        psum.tile([128, 1024], f32, tag="g", name=f"p2_{t}_{k}")
                        for k in range(KT)
                    ]
                    for grp in GROUPS:
                        for k in range(KT):
                            l2_pre(t, k, pg2[k], h2[cur], grp)
                        for kc in range(KT):
                            for k in range(KT):
                                l2_h1(t, k, pg2[k], h1[nxt], grp, kc)
                    mlv_p = heads_mm(t - 1, h2[cur]) if t > 0 else None
                    eltwise(t, "l2", pg2, c2, h2[nxt])
                    if t > 0:
                        heads_z(t - 1, mlv_p)
                heads_z(T - 1, heads_mm(T - 1, h2[T % 2]))

    nc.compile()
    return nc


def _get_nc():
    if "nc" not in _CACHE:
        _CACHE["nc"] = _build_bass()
    return _CACHE["nc"]


def kernel(z_post, eps, W_ih1, W_hh1, b_ih1, b_hh1, W_ih2, W_hh2, b_ih2, b_hh2,
           W_mean, b_mean, W_logvar, b_logvar):
    z_post = np.asarray(z_post, np.float32)
    eps = np.asarray(eps, np.float32)
    W_ih1 = np.asarray(W_ih1, np.float32)
    W_hh1 = np.asarray(W_hh1, np.float32)
    W_ih2 = np.asarray(W_ih2, np.float32)
    W_hh2 = np.asarray(W_hh2, np.float32)
    W_mean = np.asarray(W_mean, np.float32)
    W_logvar = np.asarray(W_logvar, np.float32)
    b1 = (np.asarray(b_ih1, np.float32) + np.asarray(b_hh1, np.float32))
    b2 = (np.asarray(b_ih2, np.float32) + np.asarray(b_hh2, np.float32))
    b_mean = np.asarray(b_mean, np.float32)
    b_logvar = np.asarray(b_logvar, np.float32)

    # shared (replicated) weight tensors, pre-transposed for the PE array
    w1x = np.ascontiguousarray(
        np.concatenate([W_ih1.T, b1[None, :]], 0)
    ).astype(BF)                                          # [Z+1, G]
    whh1 = np.ascontiguousarray(W_hh1.T).astype(BF)       # [H, G]
    wih2 = np.ascontiguousarray(W_ih2.T).astype(BF)
    whh2 = np.ascontiguousarray(W_hh2.T).astype(BF)
    whd = np.ascontiguousarray(
        np.concatenate([W_mean, W_logvar], 0).T
    ).astype(BF)                                          # [H, 2Z]
    b2v = np.zeros((Z + 1, G), np.float32)
    b2v[Z] = b2
    b2v = b2v.astype(BF)
    bhd = np.zeros((Z + 1, 2 * Z), np.float32)
    bhd[Z] = np.concatenate([b_mean, b_logvar])
    bhd = bhd.astype(BF)

    # x_t is the previous frame's z_post
    x_seq = np.concatenate(
        [np.zeros_like(z_post[:, :1]), z_post[:, : T - 1]], 1
    )                                                     # [B, T, Z]

    in_maps = []
    for ci in range(NCORES):
        sl = slice(ci * BL, (ci + 1) * BL)
        xc = np.ascontiguousarray(x_seq[sl].transpose(1, 2, 0))     # [T, Z, BL]
        xc = np.concatenate(
            [xc, np.ones((T, 1, BL), np.float32)], 1
        )                                                           # [T, Z+1, BL]
        # device SBUF layout [Z+1, T*BL], linear DMA
        xc = np.ascontiguousarray(xc.transpose(1, 0, 2)).reshape(Z + 1, T * BL).astype(BF)
        epsc = np.ascontiguousarray(eps[sl, :T].transpose(1, 2, 0)) # [T, Z, BL]
        in_maps.append(
            {
                "xT": xc,
                "epsT": epsc,
                "w1x": w1x,
                "whh1": whh1,
                "wih2": wih2,
                "whh2": whh2,
                "whd": whd,
                "b2": b2v,
                "bhd": bhd,
            }
        )

    from concourse.bass_utils import run_bass_kernel_spmd

    nc = _get_nc()
    trace = os.environ.get("KERNEL_PROFILE", "") == "1"
    res = run_bass_kernel_spmd(
        nc, in_maps, core_ids=list(range(NCORES)), trace=trace
    )
    if trace:
        _CACHE["exec_time_ns"] = res.exec_time_ns

    outs = []
    for name in ("zm", "zlv", "zz"):
        parts = [
            res.results[ci][name].transpose(2, 0, 1)  # [T,Z,BL] -> [BL,T,Z]
            for ci in range(NCORES)
        ]
        outs.append(np.ascontiguousarray(np.concatenate(parts, 0), np.float32))
    return tuple(outs)


# revision 17
# speedup vs baseline: 1.0776x; 1.0776x over previous
"""Trainium2 Bass kernel for a 2-layer LSTM motion-prior sampler (DBSE).

Computes, per time step t (B=2048, T=64, Z=64, H=512):
    h1,c1 = LSTMCell(x_t, h1, c1; W_ih1, W_hh1, b1)     x_t = z_post[:, t-1] (0 at t=0)
    h2,c2 = LSTMCell(h1, h2, c2; W_ih2, W_hh2, b2)
    m_t   = h2 @ W_mean.T + b_mean
    lv_t  = h2 @ W_logvar.T + b_logvar
    z_t   = m_t + eps_t * exp(0.5 * lv_t)
Returns (z_means, z_logvars, z_out), each [B, T, Z] float32.

Sharding: data-parallel over batch across 8 NeuronCores (256 rows each),
weights replicated, recurrence local per core, no collectives.

Device layout: everything feature-major [feature, batch] so LSTM matmul
outputs land directly in recurrence layout with zero transposes.
Matmul inputs are bf16 (fp32 PSUM accumulation); c-state and all
elementwise math stay fp32. Biases are folded into the matmuls (ones-row
trick for layer 1, K=1 bias matmuls for layer 2 / heads).

exp(0.5*lv) is computed in-loop as s/(1-s) with s = sigmoid(0.5*lv) so
the ScalarEngine keeps a single sigmoid/tanh activation-table set for
the whole kernel (an exp table switch costs ~2.7us and would serialize).

PSUM: one 2-bank tile per hidden k-tile ([i|f|o|g]*256), 3 rotating
slots, plus a dedicated 1-bank pool for the mean|logvar head. All
biases enter as K=65 matmuls against the [x;1] rhs with zero-padded
weight rows (K=1 bias matmuls cost a ~400ns PE pipeline break each).
Layer-2 is emitted h2-source-major with the h1-dependent k-slices last
and kc-major across tiles, so the PE absorbs the layer-1 elementwise
latency instead of stalling.
"""

import os
import sys

for _p in ("/opt/trn_rl_repo", "/root/.axon_site/_ro/trn_rl_repo"):
    if os.path.isdir(_p) and _p not in sys.path:
        sys.path.insert(0, _p)

import numpy as np
import ml_dtypes

B = 2048
T = int(os.environ.get("KERNEL_T", "64"))
Z = 64
H = 512
G = 4 * H           # 2048 gate units per layer
NCORES = 8
BL = B // NCORES    # 256 batch rows per core
KT = H // 128       # 4 contraction k-tiles of 128
BF = ml_dtypes.bfloat16

_CACHE = {}


def _build_bass():
    import concourse.tile as tile
    from concourse import bacc, mybir

    f32 = mybir.dt.float32
    bf = mybir.dt.bfloat16
    AF = mybir.ActivationFunctionType
    OP = mybir.AluOpType

    nc = bacc.Bacc(None, target_bir_lowering=False)

    # ---------------- DRAM I/O (per core) ----------------
    xT = nc.dram_tensor("xT", [Z + 1, T * BL], bf, kind="ExternalInput")
    epsT = nc.dram_tensor("epsT", [T, Z, BL], f32, kind="ExternalInput")
    w1x_d = nc.dram_tensor("w1x", [Z + 1, G], bf, kind="ExternalInput")
    whh1_d = nc.dram_tensor("whh1", [H, G], bf, kind="ExternalInput")
    wih2_d = nc.dram_tensor("wih2", [H, G], bf, kind="ExternalInput")
    whh2_d = nc.dram_tensor("whh2", [H, G], bf, kind="ExternalInput")
    whd_d = nc.dram_tensor("whd", [H, 2 * Z], bf, kind="ExternalInput")
    b2_d = nc.dram_tensor("b2", [Z + 1, G], bf, kind="ExternalInput")
    bhd_d = nc.dram_tensor("bhd", [Z + 1, 2 * Z], bf, kind="ExternalInput")
    zm_d = nc.dram_tensor("zm", [T, Z, BL], f32, kind="ExternalOutput")
    zlv_d = nc.dram_tensor("zlv", [T, Z, BL], f32, kind="ExternalOutput")
    zz_d = nc.dram_tensor("zz", [T, Z, BL], f32, kind="ExternalOutput")

    # psum chunk position -> weight-column base; gates ordered [i,f,o | g]
    # in PSUM so one sigmoid covers [0:1536] and one tanh covers [1536:2048].
    # PyTorch weight rows are [i, f, g, o] blocks of H.
    GBASE = [0 * H, 1 * H, 3 * H, 2 * H]  # i, f, o, g

    with tile.TileContext(nc) as tc:
        from contextlib import ExitStack

        with ExitStack() as ctx:
            state = ctx.enter_context(tc.tile_pool(name="state", bufs=1))

            # ---------------- persistent SBUF ----------------
            # order matters: the t=0 layer-1 matmuls only need w1x/x/whh1,
            # so those transfers go first and the loop starts sooner.
            w1x = state.tile([Z + 1, G], bf)
            nc.sync.dma_start(w1x[:], w1x_d[:])
            xall = state.tile([Z + 1, T, BL], bf)
            nc.sync.dma_start(xall[:], xT.rearrange("p (t b) -> p t b", t=T))
            whh1 = state.tile([128, KT, G], bf)
            for kc in range(KT):
                eng = nc.sync if kc % 2 == 0 else nc.scalar
                eng.dma_start(whh1[:, kc, :], whh1_d[kc * 128 : (kc + 1) * 128, :])
            whh2 = state.tile([128, KT, G], bf)
            for kc in range(KT):
                eng = nc.sync if kc % 2 == 0 else nc.scalar
                eng.dma_start(whh2[:, kc, :], whh2_d[kc * 128 : (kc + 1) * 128, :])
            wih2 = state.tile([128, KT, G], bf)
            for kc in range(KT):
                eng = nc.sync if kc % 2 == 0 else nc.scalar
                eng.dma_start(wih2[:, kc, :], wih2_d[kc * 128 : (kc + 1) * 128, :])
            whd = state.tile([128, KT, 2 * Z], bf)
            nc.sync.dma_start(whd[:], whd_d.rearrange("(k p) g -> p k g", p=128))
            b2 = state.tile([Z + 1, G], bf)
            nc.sync.dma_start(b2[:], b2_d[:])
            bhd = state.tile([Z + 1, 2 * Z], bf)
            nc.sync.dma_start(bhd[:], bhd_d[:])

            # double-buffered h (bf16, matmul input); single-buffer c (fp32)
            h1b0 = state.tile([128, KT * BL], bf)
            h1b1 = state.tile([128, KT * BL], bf)
            h2b0 = state.tile([128, KT * BL], bf)
            h2b1 = state.tile([128, KT * BL], bf)
            c1 = state.tile([128, KT * BL], f32)
            c2 = state.tile([128, KT * BL], f32)
            for buf in (h1b0, h1b1, h2b0, h2b1):
                nc.gpsimd.memset(buf[:], 0.0)
            nc.gpsimd.memset(c1[:], 0.0)
            nc.gpsimd.memset(c2[:], 0.0)
            h1 = (h1b0, h1b1)
            h2 = (h2b0, h2b1)

            with (
                tc.tile_pool(name="psum", bufs=3, space="PSUM") as psum,
                tc.tile_pool(name="psum_hd", bufs=2, space="PSUM") as psum_hd,
                tc.tile_pool(name="work", bufs=3) as work,
                tc.tile_pool(name="epsp", bufs=4) as epsp,
            ):
                # One PSUM tile per hidden k-tile (2 banks): [i|f|o|g]*256.
                # Bank A = [0:512] (i,f), bank B = [512:1024] (o,g), so the
                # start-matmul pairs (i,o) then (f,g) are bank-disjoint and
                # row-strip-packed K=1 bias matmuls run concurrently.
                POS_OFF = {"i": 0, "f": 256, "o": 512, "g": 768}
                POS_COL = {"i": 0, "f": H, "o": 3 * H, "g": 2 * H}
                POS_ROW = {"i": 0, "o": 32, "f": 64, "g": 96}
                GROUPS = (("i", "o"), ("f", "g"))

                def l1_group(t, k, pg, h_cur, grp):
                    """x-gate starts + W_hh1 accumulation for chunk pair grp
                    of k-tile k (layer 1)."""
                    for pos in grp:
                        off = POS_OFF[pos]
                        col = POS_COL[pos] + k * 128
                        nc.tensor.matmul(
                            pg[:, off : off + 256],
                            w1x[:, col : col + 128],
                            xall[:, t, :],
                            start=True,
                            stop=False,
                        )
                    for kc in range(KT):
                        for pos in grp:
                            off = POS_OFF[pos]
                            col = POS_COL[pos] + k * 128
                            nc.tensor.matmul(
                                pg[:, off : off + 256],
                                whh1[:, kc, col : col + 128],
                                h_cur[:, kc * 256 : (kc + 1) * 256],
                                start=False,
                                stop=(kc == KT - 1),
                            )

                def l2_pre(t, k, pg, h2_cur, grp):
                    """row-packed K=1 bias starts + the h2-dependent
                    accumulation for chunk pair grp of k-tile k (layer 2)."""
                    for pos in grp:
                        off = POS_OFF[pos]
                        col = POS_COL[pos] + k * 128
                        nc.tensor.matmul(
                            pg[:, off : off + 256],
                            b2[:, col : col + 128],
                            xall[:, t, :],
                            start=True,
                            stop=False,
                        )
                    for kc in range(KT):
                        for pos in grp:
                            off = POS_OFF[pos]
                            col = POS_COL[pos] + k * 128
                            nc.tensor.matmul(
                                pg[:, off : off + 256],
                                whh2[:, kc, col : col + 128],
                                h2_cur[:, kc * 256 : (kc + 1) * 256],
                                start=False,
                                stop=False,
                            )

                def l2_h1(t, k, pg, h1_new, grp, kc):
                    """one h1-dependent k-slice for chunk pair grp; emitted
                    kc-major across tiles so each h1 k-tile is consumed as
                    soon as the layer-1 elementwise produces it."""
                    for pos in grp:
                        off = POS_OFF[pos]
                        col = POS_COL[pos] + k * 128
                        nc.tensor.matmul(
                            pg[:, off : off + 256],
                            wih2[:, kc, col : col + 128],
                            h1_new[:, kc * 256 : (kc + 1) * 256],
                            start=False,
                            stop=(kc == KT - 1),
                        )

                def eltwise(t, lname, pgs, c, h_new):
                    """Per-k-tile LSTM cell elementwise, software-pipelined:
                    ACT order s0 g0 s1 g1 th0 s2 g2 th1 s3 g3 th2 th3 keeps
                    the scalar engine busy while the DVE runs the c-updates,
                    and each h k-tile is published as early as possible."""
                    ifo = [None] * KT
                    gg = [None] * KT
                    th = [None] * KT

                    def h_mul(k):
                        nc.vector.tensor_mul(
                            h_new[:, k * 256 : (k + 1) * 256],
                            ifo[k][:, 512:768],
                            th[k][:],
                        )

                    def tanh_c(k):
                        th[k] = work.tile(
                            [128, 256], bf, tag="th", name=f"th_{lname}_{t}_{k}"
                        )
                        nc.scalar.activation(
                            th[k][:], c[:, k * 256 : (k + 1) * 256], AF.Tanh
                        )

                    for k in range(KT):
                        ifo[k] = work.tile(
                            [128, 768], f32, tag="ifo", name=f"ifo_{lname}_{t}_{k}"
                        )
                        nc.scalar.activation(ifo[k][:], pgs[k][:, 0:768], AF.Sigmoid)
                        gg[k] = work.tile(
                            [128, 256], f32, tag="gg", name=f"gg_{lname}_{t}_{k}"
                        )
                        nc.scalar.activation(gg[k][:], pgs[k][:, 768:1024], AF.Tanh)
                        if k >= 2:
                            h_mul(k - 2)
                        cs = c[:, k * 256 : (k + 1) * 256]
                        t1 = work.tile([128, 256], f32, tag="t1", name=f"t1_{lname}_{t}_{k}")
                        nc.vector.tensor_mul(t1[:], ifo[k][:, 256:512], cs)
                        t2 = work.tile([128, 256], f32, tag="t2", name=f"t2_{lname}_{t}_{k}")
                        nc.vector.tensor_mul(t2[:], ifo[k][:, 0:256], gg[k][:])
                        nc.vector.tensor_add(cs, t1[:], t2[:])
                        if k >= 1:
                            tanh_c(k - 1)
                    tanh_c(KT - 1)
                    h_mul(KT - 2)
                    h_mul(KT - 1)

                def heads_mm(t, h2_new):
                    """mean|logvar head matmuls for step t -> SBUF copy."""
                    ph = psum_hd.tile([128, BL], f32, tag="hd", name=f"ph_{t}")
                    nc.tensor.matmul(
                        ph[:, 0:BL], bhd[:], xall[:, t, :], start=True, stop=False
                    )
                    for kc in range(KT):
                        nc.tensor.matmul(
                            ph[:, 0:BL],
                            whd[:, kc, :],
                            h2_new[:, kc * 256 : (kc + 1) * 256],
                            start=False,
                            stop=(kc == KT - 1),
                        )
                    mlv = work.tile([128, BL], f32, tag="mlv", name=f"mlv_{t}")
                    nc.vector.tensor_copy(mlv[:], ph[:, 0:BL])
                    return mlv

                def heads_z(t, mlv):
                    """z_t = m + eps * exp(0.5*lv), with exp via the
                    sigmoid table set: exp(x) = s/(1-s), s = sigmoid(x/2).
                    lv lives on partitions 64..127; a small SBUF->SBUF DMA
                    aligns it with m on partitions 0..63."""
                    lvs = work.tile([Z, BL], f32, tag="lvs", name=f"lvs_{t}")
                    nc.sync.dma_start(lvs[:], mlv[Z : 2 * Z, :])
                    s = work.tile([Z, BL], f32, tag="s", name=f"s_{t}")
                    nc.scalar.activation(s[:], lvs[:], AF.Sigmoid, scale=0.5)
                    u = work.tile([Z, BL], f32, tag="u", name=f"u_{t}")
                    nc.vector.tensor_scalar(u[:], s[:], -1.0, 1.0, OP.mult, OP.add)
                    r = work.tile([Z, BL], f32, tag="r", name=f"r_{t}")
                    nc.vector.reciprocal(r[:], u[:])
                    e = work.tile([Z, BL], f32, tag="e", name=f"e_{t}")
                    nc.vector.tensor_mul(e[:], s[:], r[:])
                    epst = epsp.tile([Z, BL], f32, tag="eps", name=f"eps_{t}")
                    nc.sync.dma_start(epst[:], epsT[t])
                    zt = work.tile([Z, BL], f32, tag="zt", name=f"zt_{t}")
                    nc.vector.tensor_mul(zt[:], e[:], epst[:])
                    nc.vector.tensor_add(zt[:], zt[:], mlv[0:Z, :])
                    nc.sync.dma_start(zm_d[t], mlv[0:Z, :])
                    nc.sync.dma_start(zlv_d[t], lvs[:])
                    nc.sync.dma_start(zz_d[t], zt[:])

                # ---------------- recurrence ----------------
                for t in range(T):
                    cur, nxt = t % 2, (t + 1) % 2
                    pg1 = [
                        psum.tile([128, 1024], f32, tag="g", name=f"p1_{t}_{k}")
                        for k in range(KT)
                    ]
                    for k in range(KT):
                        for grp in GROUPS:
                            l1_group(t, k, pg1[k], h1[cur], grp)
                    eltwise(t, "l1", pg1, c1, h1[nxt])
                    pg2 = [
                        psum.tile([128, 1024], f32, tag="g", name=f"p2_{t}_{k}")
                        for k in range(KT)
                    ]
                    for grp in GROUPS:
                        for k in range(KT):
                            l2_pre(t, k, pg2[k], h2[cur], grp)
                        for kc in range(KT):
                            for k in range(KT):
                                l2_h1(t, k, pg2[k], h1[nxt], grp, kc)
                    eltwise(t, "l2", pg2, c2, h2[nxt])
                    if t > 0:
                        heads_z(t - 1, heads_mm(t - 1, h2[cur]))
                heads_z(T - 1, heads_mm(T - 1, h2[T % 2]))

    nc.compile()
    return nc


def _get_nc():
    if "nc" not in _CACHE:
        _CACHE["nc"] = _build_bass()
    return _CACHE["nc"]


def kernel(z_post, eps, W_ih1, W_hh1, b_ih1, b_hh1, W_ih2, W_hh2, b_ih2, b_hh2,
           W_mean, b_mean, W_logvar, b_logvar):
    z_post = np.asarray(z_post, np.float32)
    eps = np.asarray(eps, np.float32)
    W_ih1 = np.asarray(W_ih1, np.float32)
    W_hh1 = np.asarray(W_hh1, np.float32)
    W_ih2 = np.asarray(W_ih2, np.float32)
    W_hh2 = np.asarray(W_hh2, np.float32)
    W_mean = np.asarray(W_mean, np.float32)
    W_logvar = np.asarray(W_logvar, np.float32)
    b1 = (np.asarray(b_ih1, np.float32) + np.asarray(b_hh1, np.float32))
    b2 = (np.asarray(b_ih2, np.float32) + np.asarray(b_hh2, np.float32))
    b_mean = np.asarray(b_mean, np.float32)
    b_logvar = np.asarray(b_logvar, np.float32)

    # shared (replicated) weight tensors, pre-transposed for the PE array
    w1x = np.ascontiguousarray(
        np.concatenate([W_ih1.T, b1[None, :]], 0)
    ).astype(BF)                                          # [Z+1, G]
    whh1 = np.ascontiguousarray(W_hh1.T).astype(BF)       # [H, G]
    wih2 = np.ascontiguousarray(W_ih2.T).astype(BF)
    whh2 = np.ascontiguousarray(W_hh2.T).astype(BF)
    whd = np.ascontiguousarray(
        np.concatenate([W_mean, W_logvar], 0).T
    ).astype(BF)                                          # [H, 2Z]
    b2v = np.zeros((Z + 1, G), np.float32)
    b2v[Z] = b2
    b2v = b2v.astype(BF)
    bhd = np.zeros((Z + 1, 2 * Z), np.float32)
    bhd[Z] = np.concatenate([b_mean, b_logvar])
    bhd = bhd.astype(BF)

    # x_t is the previous frame's z_post
    x_seq = np.concatenate(
        [np.zeros_like(z_post[:, :1]), z_post[:, : T - 1]], 1
    )                                                     # [B, T, Z]

    in_maps = []
    for ci in range(NCORES):
        sl = slice(ci * BL, (ci + 1) * BL)
        xc = np.ascontiguousarray(x_seq[sl].transpose(1, 2, 0))     # [T, Z, BL]
        xc = np.concatenate(
            [xc, np.ones((T, 1, BL), np.float32)], 1
        )                                                           # [T, Z+1, BL]
        # device SBUF layout [Z+1, T*BL], linear DMA
        xc = np.ascontiguousarray(xc.transpose(1, 0, 2)).reshape(Z + 1, T * BL).astype(BF)
        epsc = np.ascontiguousarray(eps[sl, :T].transpose(1, 2, 0)) # [T, Z, BL]
        in_maps.append(
            {
                "xT": xc,
                "epsT": epsc,
                "w1x": w1x,
                "whh1": whh1,
                "wih2": wih2,
                "whh2": whh2,
                "whd": whd,
                "b2": b2v,
                "bhd": bhd,
            }
        )

    from concourse.bass_utils import run_bass_kernel_spmd

    nc = _get_nc()
    trace = os.environ.get("KERNEL_PROFILE", "") == "1"
    res = run_bass_kernel_spmd(
        nc, in_maps, core_ids=list(range(NCORES)), trace=trace
    )
    if trace:
        _CACHE["exec_time_ns"] = res.exec_time_ns

    outs = []
    for name in ("zm", "zlv", "zz"):
        parts = [
            res.results[ci][name].transpose(2, 0, 1)  # [T,Z,BL] -> [BL,T,Z]
            for ci in range(NCORES)
        ]
        outs.append(np.ascontiguousarray(np.concatenate(parts, 0), np.float32))
    return tuple(outs)


# revision 18
# speedup vs baseline: 1.1264x; 1.0453x over previous
"""Trainium2 Bass kernel for a 2-layer LSTM motion-prior sampler (DBSE).

Computes, per time step t (B=2048, T=64, Z=64, H=512):
    h1,c1 = LSTMCell(x_t, h1, c1; W_ih1, W_hh1, b1)     x_t = z_post[:, t-1] (0 at t=0)
    h2,c2 = LSTMCell(h1, h2, c2; W_ih2, W_hh2, b2)
    m_t   = h2 @ W_mean.T + b_mean
    lv_t  = h2 @ W_logvar.T + b_logvar
    z_t   = m_t + eps_t * exp(0.5 * lv_t)
Returns (z_means, z_logvars, z_out), each [B, T, Z] float32.

Sharding: data-parallel over batch across 8 NeuronCores (256 rows each),
weights replicated, recurrence local per core, no collectives.

Device layout: everything feature-major [feature, batch] so LSTM matmul
outputs land directly in recurrence layout with zero transposes.
Matmul inputs are bf16 (fp32 PSUM accumulation); c-state and all
elementwise math stay fp32. Biases are folded into the matmuls (ones-row
trick for layer 1, K=1 bias matmuls for layer 2 / heads).

exp(0.5*lv) is computed in-loop as s/(1-s) with s = sigmoid(0.5*lv) so
the ScalarEngine keeps a single sigmoid/tanh activation-table set for
the whole kernel (an exp table switch costs ~2.7us and would serialize).

PSUM: one 2-bank tile per hidden k-tile ([i|f|o|g]*256), 3 rotating
slots, plus a dedicated 1-bank pool for the mean|logvar head. All
biases enter as K=65 matmuls against the [x;1] rhs with zero-padded
weight rows (K=1 bias matmuls cost a ~400ns PE pipeline break each).
Layer-2 is emitted h2-source-major with the h1-dependent k-slices last
and kc-major across tiles, so the PE absorbs the layer-1 elementwise
latency instead of stalling.
"""

import os
import sys

for _p in ("/opt/trn_rl_repo", "/root/.axon_site/_ro/trn_rl_repo"):
    if os.path.isdir(_p) and _p not in sys.path:
        sys.path.insert(0, _p)

import numpy as np
import ml_dtypes

B = 2048
T = int(os.environ.get("KERNEL_T", "64"))
Z = 64
H = 512
G = 4 * H           # 2048 gate units per layer
NCORES = 8
BL = B // NCORES    # 256 batch rows per core
KT = H // 128       # 4 contraction k-tiles of 128
BF = ml_dtypes.bfloat16

_CACHE = {}


def _build_bass():
    import concourse.tile as tile
    from concourse import bacc, mybir

    f32 = mybir.dt.float32
    bf = mybir.dt.bfloat16
    AF = mybir.ActivationFunctionType
    OP = mybir.AluOpType

    nc = bacc.Bacc(None, target_bir_lowering=False)

    # ---------------- DRAM I/O (per core) ----------------
    xT = nc.dram_tensor("xT", [Z + 1, T * BL], bf, kind="ExternalInput")
    epsT = nc.dram_tensor("epsT", [T, Z, BL], f32, kind="ExternalInput")
    w1x_d = nc.dram_tensor("w1x", [Z + 1, G], bf, kind="ExternalInput")
    whh1_d = nc.dram_tensor("whh1", [H, G], bf, kind="ExternalInput")
    wih2_d = nc.dram_tensor("wih2", [H, G], bf, kind="ExternalInput")
    whh2_d = nc.dram_tensor("whh2", [H, G], bf, kind="ExternalInput")
    whd_d = nc.dram_tensor("whd", [H, 2 * Z], bf, kind="ExternalInput")
    b2_d = nc.dram_tensor("b2", [Z + 1, G], bf, kind="ExternalInput")
    bhd_d = nc.dram_tensor("bhd", [Z + 1, 2 * Z], bf, kind="ExternalInput")
    zm_d = nc.dram_tensor("zm", [T, Z, BL], f32, kind="ExternalOutput")
    zlv_d = nc.dram_tensor("zlv", [T, Z, BL], f32, kind="ExternalOutput")
    zz_d = nc.dram_tensor("zz", [T, Z, BL], f32, kind="ExternalOutput")

    # psum chunk position -> weight-column base; gates ordered [i,f,o | g]
    # in PSUM so one sigmoid covers [0:1536] and one tanh covers [1536:2048].
    # PyTorch weight rows are [i, f, g, o] blocks of H.
    GBASE = [0 * H, 1 * H, 3 * H, 2 * H]  # i, f, o, g

    with tile.TileContext(nc) as tc:
        from contextlib import ExitStack

        with ExitStack() as ctx:
            state = ctx.enter_context(tc.tile_pool(name="state", bufs=1))

            # ---------------- persistent SBUF ----------------
            # order matters: the t=0 layer-1 matmuls only need w1x/x/whh1,
            # so those transfers go first and the loop starts sooner.
            w1x = state.tile([Z + 1, G], bf)
            nc.sync.dma_start(w1x[:], w1x_d[:])
            xall = state.tile([Z + 1, T, BL], bf)
            nc.sync.dma_start(xall[:], xT.rearrange("p (t b) -> p t b", t=T))
            whh1 = state.tile([128, KT, G], bf)
            for kc in range(KT):
                eng = nc.sync if kc % 2 == 0 else nc.scalar
                eng.dma_start(whh1[:, kc, :], whh1_d[kc * 128 : (kc + 1) * 128, :])
            whh2 = state.tile([128, KT, G], bf)
            for kc in range(KT):
                eng = nc.sync if kc % 2 == 0 else nc.scalar
                eng.dma_start(whh2[:, kc, :], whh2_d[kc * 128 : (kc + 1) * 128, :])
            wih2 = state.tile([128, KT, G], bf)
            for kc in range(KT):
                eng = nc.sync if kc % 2 == 0 else nc.scalar
                eng.dma_start(wih2[:, kc, :], wih2_d[kc * 128 : (kc + 1) * 128, :])
            whd = state.tile([128, KT, 2 * Z], bf)
            nc.sync.dma_start(whd[:], whd_d.rearrange("(k p) g -> p k g", p=128))
            b2 = state.tile([Z + 1, G], bf)
            nc.sync.dma_start(b2[:], b2_d[:])
            bhd = state.tile([Z + 1, 2 * Z], bf)
            nc.sync.dma_start(bhd[:], bhd_d[:])

            # double-buffered h (bf16, matmul input); single-buffer c (fp32)
            h1b0 = state.tile([128, KT * BL], bf)
            h1b1 = state.tile([128, KT * BL], bf)
            h2b0 = state.tile([128, KT * BL], bf)
            h2b1 = state.tile([128, KT * BL], bf)
            c1 = state.tile([128, KT * BL], f32)
            c2 = state.tile([128, KT * BL], f32)
            for buf in (h1b0, h1b1, h2b0, h2b1):
                nc.gpsimd.memset(buf[:], 0.0)
            nc.gpsimd.memset(c1[:], 0.0)
            nc.gpsimd.memset(c2[:], 0.0)
            h1 = (h1b0, h1b1)
            h2 = (h2b0, h2b1)

            with (
                tc.tile_pool(name="psum", bufs=3, space="PSUM") as psum,
                tc.tile_pool(name="psum_hd", bufs=2, space="PSUM") as psum_hd,
                tc.tile_pool(name="work", bufs=3) as work,
                tc.tile_pool(name="epsp", bufs=4) as epsp,
            ):
                # One PSUM tile per hidden k-tile (2 banks): [i|f|o|g]*256.
                # Bank A = [0:512] (i,f), bank B = [512:1024] (o,g), so the
                # start-matmul pairs (i,o) then (f,g) are bank-disjoint and
                # row-strip-packed K=1 bias matmuls run concurrently.
                POS_OFF = {"i": 0, "f": 256, "o": 512, "g": 768}
                POS_COL = {"i": 0, "f": H, "o": 3 * H, "g": 2 * H}
                POS_ROW = {"i": 0, "o": 32, "f": 64, "g": 96}
                GROUPS = (("i", "o"), ("f", "g"))

                def l1_group(t, k, pg, h_cur, grp):
                    """x-gate starts + W_hh1 accumulation for chunk pair grp
                    of k-tile k (layer 1)."""
                    for pos in grp:
                        off = POS_OFF[pos]
                        col = POS_COL[pos] + k * 128
                        nc.tensor.matmul(
                            pg[:, off : off + 256],
                            w1x[:, col : col + 128],
                            xall[:, t, :],
                            start=True,
                            stop=False,
                        )
                    for kc in range(KT):
                        for pos in grp:
                            off = POS_OFF[pos]
                            col = POS_COL[pos] + k * 128
                            nc.tensor.matmul(
                                pg[:, off : off + 256],
                                whh1[:, kc, col : col + 128],
                                h_cur[:, kc * 256 : (kc + 1) * 256],
                                start=False,
                                stop=(kc == KT - 1),
                            )

                def l2_pre(t, k, pg, h2_cur, grp):
                    """row-packed K=1 bias starts + the h2-dependent
                    accumulation for chunk pair grp of k-tile k (layer 2)."""
                    for pos in grp:
                        off = POS_OFF[pos]
                        col = POS_COL[pos] + k * 128
                        nc.tensor.matmul(
                            pg[:, off : off + 256],
                            b2[:, col : col + 128],
                            xall[:, t, :],
                            start=True,
                            stop=False,
                        )
                    for kc in range(KT):
                        for pos in grp:
                            off = POS_OFF[pos]
                            col = POS_COL[pos] + k * 128
                            nc.tensor.matmul(
                                pg[:, off : off + 256],
                                whh2[:, kc, col : col + 128],
                                h2_cur[:, kc * 256 : (kc + 1) * 256],
                                start=False,
                                stop=False,
                            )

                def l2_h1(t, k, pg, h1_new, grp, kc):
                    """one h1-dependent k-slice for chunk pair grp; emitted
                    kc-major across tiles so each h1 k-tile is consumed as
                    soon as the layer-1 elementwise produces it."""
                    for pos in grp:
                        off = POS_OFF[pos]
                        col = POS_COL[pos] + k * 128
                        nc.tensor.matmul(
                            pg[:, off : off + 256],
                            wih2[:, kc, col : col + 128],
                            h1_new[:, kc * 256 : (kc + 1) * 256],
                            start=False,
                            stop=(kc == KT - 1),
                        )

                def eltwise(t, lname, pgs, c, h_new):
                    """Per-k-tile LSTM cell elementwise, software-pipelined:
                    ACT order s0 g0 s1 g1 th0 s2 g2 th1 s3 g3 th2 th3 keeps
                    the scalar engine busy while the DVE runs the c-updates,
                    and each h k-tile is published as early as possible."""
                    ifo = [None] * KT
                    gg = [None] * KT
                    th = [None] * KT

                    def h_mul(k):
                        nc.vector.tensor_mul(
                            h_new[:, k * 256 : (k + 1) * 256],
                            ifo[k][:, 512:768],
                            th[k][:],
                        )

                    def tanh_c(k):
                        th[k] = work.tile(
                            [128, 256], bf, tag="th", name=f"th_{lname}_{t}_{k}"
                        )
                        nc.scalar.activation(
                            th[k][:], c[:, k * 256 : (k + 1) * 256], AF.Tanh
                        )

                    for k in range(KT):
                        ifo[k] = work.tile(
                            [128, 768], f32, tag="ifo", name=f"ifo_{lname}_{t}_{k}"
                        )
                        nc.scalar.activation(ifo[k][:], pgs[k][:, 0:768], AF.Sigmoid)
                        gg[k] = work.tile(
                            [128, 256], f32, tag="gg", name=f"gg_{lname}_{t}_{k}"
                        )
                        nc.scalar.activation(gg[k][:], pgs[k][:, 768:1024], AF.Tanh)
                        if k >= 2:
                            h_mul(k - 2)
                        cs = c[:, k * 256 : (k + 1) * 256]
                        t1 = work.tile([128, 256], f32, tag="t1", name=f"t1_{lname}_{t}_{k}")
                        nc.vector.tensor_mul(t1[:], ifo[k][:, 256:512], cs)
                        t2 = work.tile([128, 256], f32, tag="t2", name=f"t2_{lname}_{t}_{k}")
                        nc.vector.tensor_mul(t2[:], ifo[k][:, 0:256], gg[k][:])
                        nc.vector.tensor_add(cs, t1[:], t2[:])
                        if k >= 1:
                            tanh_c(k - 1)
                    tanh_c(KT - 1)
                    h_mul(KT - 2)
                    h_mul(KT - 1)

                def heads_mm(t, h2_new):
                    """mean|logvar head matmuls for step t -> SBUF copy."""
                    ph = psum_hd.tile([128, BL], f32, tag="hd", name=f"ph_{t}")
                    nc.tensor.matmul(
                        ph[:, 0:BL], bhd[:], xall[:, t, :], start=True, stop=False
                    )
                    for kc in range(KT):
                        nc.tensor.matmul(
                            ph[:, 0:BL],
                            whd[:, kc, :],
                            h2_new[:, kc * 256 : (kc + 1) * 256],
                            start=False,
                            stop=(kc == KT - 1),
                        )
                    mlv = work.tile([128, BL], f32, tag="mlv", name=f"mlv_{t}")
                    nc.vector.tensor_copy(mlv[:], ph[:, 0:BL])
                    return mlv

                def heads_z(t, mlv):
                    """z_t = m + eps * exp(0.5*lv), with exp via the
                    sigmoid table set: exp(x) = s/(1-s), s = sigmoid(x/2).
                    lv lives on partitions 64..127; a small SBUF->SBUF DMA
                    aligns it with m on partitions 0..63."""
                    lvs = work.tile([Z, BL], f32, tag="lvs", name=f"lvs_{t}")
                    nc.sync.dma_start(lvs[:], mlv[Z : 2 * Z, :])
                    s = work.tile([Z, BL], f32, tag="s", name=f"s_{t}")
                    nc.scalar.activation(s[:], lvs[:], AF.Sigmoid, scale=0.5)
                    u = work.tile([Z, BL], f32, tag="u", name=f"u_{t}")
                    nc.vector.tensor_scalar(u[:], s[:], -1.0, 1.0, OP.mult, OP.add)
                    r = work.tile([Z, BL], f32, tag="r", name=f"r_{t}")
                    nc.vector.reciprocal(r[:], u[:])
                    e = work.tile([Z, BL], f32, tag="e", name=f"e_{t}")
                    nc.vector.tensor_mul(e[:], s[:], r[:])
                    epst = epsp.tile([Z, BL], f32, tag="eps", name=f"eps_{t}")
                    nc.sync.dma_start(epst[:], epsT[t])
                    zt = work.tile([Z, BL], f32, tag="zt", name=f"zt_{t}")
                    nc.vector.tensor_mul(zt[:], e[:], epst[:])
                    nc.vector.tensor_add(zt[:], zt[:], mlv[0:Z, :])
                    nc.sync.dma_start(zm_d[t], mlv[0:Z, :])
                    nc.sync.dma_start(zlv_d[t], lvs[:])
                    nc.sync.dma_start(zz_d[t], zt[:])

                # ---------------- recurrence ----------------
                for t in range(T):
                    cur, nxt = t % 2, (t + 1) % 2
                    pg1 = [
                        psum.tile([128, 1024], f32, tag="g", name=f"p1_{t}_{k}")
                        for k in range(KT)
                    ]
                    for k in range(KT):
                        for grp in GROUPS:
                            l1_group(t, k, pg1[k], h1[cur], grp)
                    eltwise(t, "l1", pg1, c1, h1[nxt])
                    pg2 = [
                        psum.tile([128, 1024], f32, tag="g", name=f"p2_{t}_{k}")
                        for k in range(KT)
                    ]
                    for gi, grp in enumerate(GROUPS):
                        for k in range(KT):
                            l2_pre(t, k, pg2[k], h2[cur], grp)
                        if gi == 0:
                            # kc-major across tiles: consumes each h1 k-tile
                            # the moment the layer-1 elementwise publishes it
                            for kc in range(KT):
                                for k in range(KT):
                                    l2_h1(t, k, pg2[k], h1[nxt], grp, kc)
                        else:
                            # tile-major: early tiles complete sooner, so
                            # their PSUM slots release before step t+1's
                            # layer-1 matmuls need them
                            for k in range(KT):
                                for kc in range(KT):
                                    l2_h1(t, k, pg2[k], h1[nxt], grp, kc)
                    eltwise(t, "l2", pg2, c2, h2[nxt])
                    if t > 0:
                        heads_z(t - 1, heads_mm(t - 1, h2[cur]))
                heads_z(T - 1, heads_mm(T - 1, h2[T % 2]))

    nc.compile()
    return nc


def _get_nc():
    if "nc" not in _CACHE:
        _CACHE["nc"] = _build_bass()
    return _CACHE["nc"]


def kernel(z_post, eps, W_ih1, W_hh1, b_ih1, b_hh1, W_ih2, W_hh2, b_ih2, b_hh2,
           W_mean, b_mean, W_logvar, b_logvar):
    z_post = np.asarray(z_post, np.float32)
    eps = np.asarray(eps, np.float32)
    W_ih1 = np.asarray(W_ih1, np.float32)
    W_hh1 = np.asarray(W_hh1, np.float32)
    W_ih2 = np.asarray(W_ih2, np.float32)
    W_hh2 = np.asarray(W_hh2, np.float32)
    W_mean = np.asarray(W_mean, np.float32)
    W_logvar = np.asarray(W_logvar, np.float32)
    b1 = (np.asarray(b_ih1, np.float32) + np.asarray(b_hh1, np.float32))
    b2 = (np.asarray(b_ih2, np.float32) + np.asarray(b_hh2, np.float32))
    b_mean = np.asarray(b_mean, np.float32)
    b_logvar = np.asarray(b_logvar, np.float32)

    # shared (replicated) weight tensors, pre-transposed for the PE array
    w1x = np.ascontiguousarray(
        np.concatenate([W_ih1.T, b1[None, :]], 0)
    ).astype(BF)                                          # [Z+1, G]
    whh1 = np.ascontiguousarray(W_hh1.T).astype(BF)       # [H, G]
    wih2 = np.ascontiguousarray(W_ih2.T).astype(BF)
    whh2 = np.ascontiguousarray(W_hh2.T).astype(BF)
    whd = np.ascontiguousarray(
        np.concatenate([W_mean, W_logvar], 0).T
    ).astype(BF)                                          # [H, 2Z]
    b2v = np.zeros((Z + 1, G), np.float32)
    b2v[Z] = b2
    b2v = b2v.astype(BF)
    bhd = np.zeros((Z + 1, 2 * Z), np.float32)
    bhd[Z] = np.concatenate([b_mean, b_logvar])
    bhd = bhd.astype(BF)

    # x_t is the previous frame's z_post
    x_seq = np.concatenate(
        [np.zeros_like(z_post[:, :1]), z_post[:, : T - 1]], 1
    )                                                     # [B, T, Z]

    in_maps = []
    for ci in range(NCORES):
        sl = slice(ci * BL, (ci + 1) * BL)
        xc = np.ascontiguousarray(x_seq[sl].transpose(1, 2, 0))     # [T, Z, BL]
        xc = np.concatenate(
            [xc, np.ones((T, 1, BL), np.float32)], 1
        )                                                           # [T, Z+1, BL]
        # device SBUF layout [Z+1, T*BL], linear DMA
        xc = np.ascontiguousarray(xc.transpose(1, 0, 2)).reshape(Z + 1, T * BL).astype(BF)
        epsc = np.ascontiguousarray(eps[sl, :T].transpose(1, 2, 0)) # [T, Z, BL]
        in_maps.append(
            {
                "xT": xc,
                "epsT": epsc,
                "w1x": w1x,
                "whh1": whh1,
                "wih2": wih2,
                "whh2": whh2,
                "whd": whd,
                "b2": b2v,
                "bhd": bhd,
            }
        )

    from concourse.bass_utils import run_bass_kernel_spmd

    nc = _get_nc()
    trace = os.environ.get("KERNEL_PROFILE", "") == "1"
    res = run_bass_kernel_spmd(
        nc, in_maps, core_ids=list(range(NCORES)), trace=trace
    )
    if trace:
        _CACHE["exec_time_ns"] = res.exec_time_ns

    outs = []
    for name in ("zm", "zlv", "zz"):
        parts = [
            res.results[ci][name].transpose(2, 0, 1)  # [T,Z,BL] -> [BL,T,Z]
            for ci in range(NCORES)
        ]
        outs.append(np.ascontiguousarray(np.concatenate(parts, 0), np.float32))
    return tuple(outs)
